# revision 13
# baseline (speedup 1.0000x reference)
"""RNN-T JointNetwork kernel for 8 Trainium2 NeuronCores.

Math: out[b,t,u,:] = tanh(concat(fe[b,t], gd[b,u])) @ Wj + bj
with fe = f@We+be, gd = g@Wd+bd.

tanh is elementwise and the concat feeds one GEMM, so the joint GEMM
factorizes exactly:
    out[b,t,u,:] = A[b,t,:] + C[b,u,:]
    A = tanh(f@We+be) @ Wj[:Dm]
    C = tanh(g@Wd+bd) @ Wj[Dm:] + bj
leaving the kernel bound by the output write (fp16: 16 MB/core).

Sharding: core c owns (b = c//2, t-half = c%2) -> [128,64,V] output chunk.

On-core plan (fp16 data, fp32 PSUM):
  - f/g straight-DMA'd, transposed on PE (fp16 psum passthrough);
    feT/gdT on PE; tanh+bias on ACT -> tfT/tgT
  - ACP[h] (h = t-half of the core's 128 rows): [128,V] fp16 tile with
    partitions 0:64 = C rows (all u) and 64:128 = A rows (t in half h),
    computed into one PSUM tile with C (cols 0:64) and A (cols 64:128)
    matmuls interleaved so the two PE column-groups run concurrently.
  - main loop, per output tile k (= t-pair 2k,2k+1; 128 DRAM rows):
    psO = L_k.T @ ACP[h], where L_k is a host-built 0/1 stationary
    selecting (A row, C row) per output row -> A-broadcast + C-add in
    a single PE pass.  Movers alternate per tile: DVE / ACT copy
    PSUM->SBUF fp16.  Output DMA: 1 MB per 4 tiles.
Host: casts inputs to fp16, builds L/identity, upcasts output on gather.
"""

import sys

sys.path.insert(0, "/opt/trn_rl_repo")

import numpy as np

import concourse.bacc as bacc
import concourse.mybir as mybir
import concourse.tile as tile
from concourse.bass_utils import run_bass_kernel_spmd
B, T, U = 4, 256, 64
D = 512  # DE = DD = DM
V = 1024
TC = 128  # t rows per core
NCORES = 8
FP32 = mybir.dt.float32
FP16 = mybir.dt.float16
TANH = mybir.ActivationFunctionType.Tanh
WIDE = False  # N=1024 matmul per tile fails the walrus ISA check (1 bank/MM)

_cache = {}
_L_CONSTS = None


def _build_nc():
    nc = bacc.Bacc("TRN2", target_bir_lowering=False)

    f_d = nc.dram_tensor("f_c", [TC, D + 256], FP16, kind="ExternalInput")
    g_d = nc.dram_tensor("g_c", [U, D], FP16, kind="ExternalInput")
    We_d = nc.dram_tensor("We", [D, D], FP16, kind="ExternalInput")
    Wd_d = nc.dram_tensor("Wd", [D, D], FP16, kind="ExternalInput")
    Wj_d = nc.dram_tensor("Wj", [2 * D, V], FP16, kind="ExternalInput")
    L_d = nc.dram_tensor("Lsel", [128, 32 * 128], FP16, kind="ExternalInput")
    bias_d = nc.dram_tensor("bias8", [128, 8], FP32, kind="ExternalInput")
    bj_d = nc.dram_tensor("bj", [1, V], FP16, kind="ExternalInput")
    out_d = nc.dram_tensor("out", [TC * U, V], FP16, kind="ExternalOutput")

    with tile.TileContext(nc) as tc:
        with tc.tile_pool(name="wts", bufs=1) as wp:
            # ---- persistent SBUF ----
            fc_sb = wp.tile([TC, D + 256], FP16, tag="f")
            f_sb = fc_sb[:, 0:D]
            c_sb = fc_sb[:, D : D + 256]
            g_sb = wp.tile([U, D], FP16, tag="g")
            We_sb = wp.tile([128, 4 * D], FP16, tag="We")
            Wd_sb = wp.tile([128, 4 * D], FP16, tag="Wd")
            Wj_sb = wp.tile([128, 8 * V], FP16, tag="Wj")
            L_sb = wp.tile([128, 32 * 128], FP16, tag="Lsel")
            bias_sb = wp.tile([128, 8], FP32, tag="bias8")
            bj_sb = wp.tile([1, V], FP16, tag="bj")
            fT = [wp.tile([128, TC], FP16, tag=f"fT{c}", name=f"fT{c}") for c in range(4)]
            gT = [wp.tile([128, U], FP16, tag=f"gT{c}", name=f"gT{c}") for c in range(4)]
            tfT = [wp.tile([128, TC], FP16, tag=f"tfT{c}", name=f"tfT{c}") for c in range(4)]
            tgT = [wp.tile([128, U], FP16, tag=f"tgT{c}", name=f"tgT{c}") for c in range(4)]
            ACP = [wp.tile([128, V], FP16, tag=f"ACP{h}", name=f"ACP{h}") for h in range(2)]

            ident = c_sb[:, 0:128]
            ones1 = c_sb[0:1, 128 : 128 + U]
            fc_sb  # noqa: B018

            # ---- input DMAs: few big transfers, split across HWDGE rings
            # sync ring: f+consts, We, Wj v-half 0, L (outputs follow in FIFO)
            # scalar ring: g, Wd, bias, bj, Wj v-half 1
            nc.sync.dma_start(fc_sb[:], f_d[:])
            nc.sync.dma_start(
                We_sb[:], We_d.rearrange("(c p) m -> p c m", p=128)
            )
            nc.scalar.dma_start(g_sb[:], g_d[:])
            nc.scalar.dma_start(
                Wd_sb[:], Wd_d.rearrange("(c p) m -> p c m", p=128)
            )
            nc.scalar.dma_start(bias_sb[:], bias_d[:])
            nc.scalar.dma_start(bj_sb[:], bj_d[:])
            # Wj_sb layout: [128, c*V + v] (chunk-major); DMA by row-half:
            # A rows (chunks 0-3) on sync, C rows (4-7) on scalar, so the
            # A matmuls can start as soon as the first MB lands.
            nc.sync.dma_start(
                Wj_sb[:, 0 : 4 * V].rearrange("p (c v) -> p c v", c=4),
                Wj_d[0 : 4 * 128, :].rearrange("(c p) v -> p c v", p=128),
            )
            nc.scalar.dma_start(
                Wj_sb[:, 4 * V : 8 * V].rearrange("p (c v) -> p c v", c=4),
                Wj_d[4 * 128 : 8 * 128, :].rearrange("(c p) v -> p c v", p=128),
            )
            # L selectors: first 8 early (scalar), rest late (sync)
            nc.scalar.dma_start(L_sb[:, 0 : 8 * 128], L_d[:, 0 : 8 * 128])
            nc.sync.dma_start(L_sb[:, 8 * 128 :], L_d[:, 8 * 128 :])

            # ---- prologue ----
            with tc.tile_pool(name="pp", bufs=4, space="PSUM") as pp:
                # transposes on PE (fp16 passthrough), copies split DVE/ACT
                for c in range(4):
                    pt = pp.tile([128, U], FP16, tag="ppt")
                    nc.tensor.transpose(
                        pt[:], g_sb[:, c * 128 : (c + 1) * 128], ident[0:64, 0:64]
                    )
                    if c % 2 == 0:
                        nc.vector.tensor_copy(gT[c][:], pt[:])
                    else:
                        nc.scalar.copy(gT[c][:], pt[:])
                for c in range(4):
                    pt = pp.tile([128, TC], FP16, tag="ppt")
                    nc.tensor.transpose(
                        pt[:], f_sb[:, c * 128 : (c + 1) * 128], ident
                    )
                    if c % 2 == 0:
                        nc.vector.tensor_copy(fT[c][:], pt[:])
                    else:
                        nc.scalar.copy(fT[c][:], pt[:])

                for mc in range(4):
                    ms = slice(mc * 128, (mc + 1) * 128)
                    ps = pp.tile([128, U], FP32, tag="pps")
                    for dc in range(4):
                        nc.tensor.matmul(
                            ps[:],
                            Wd_sb[:, dc * D : (dc + 1) * D][:, ms],
                            gT[dc][:],
                            start=(dc == 0),
                            stop=(dc == 3),
                        )
                    nc.scalar.activation(
                        tgT[mc][:], ps[:], TANH, bias=bias_sb[:, 4 + mc : 5 + mc]
                    )
                for mc in range(4):
                    ms = slice(mc * 128, (mc + 1) * 128)
                    ps = pp.tile([128, TC], FP32, tag="pps")
                    for dc in range(4):
                        nc.tensor.matmul(
                            ps[:],
                            We_sb[:, dc * D : (dc + 1) * D][:, ms],
                            fT[dc][:],
                            start=(dc == 0),
                            stop=(dc == 3),
                        )
                    nc.scalar.activation(
                        tfT[mc][:], ps[:], TANH, bias=bias_sb[:, mc : mc + 1]
                    )

                # ACP[h]: partitions 0:64 = C (all u), 64:128 = A rows of
                # half h.  A matmuls (gated on the sync Wj half) run first
                # across all four psum tiles; C matmuls follow when the
                # scalar Wj half lands; copies fire per tile as C completes.
                Wjc = lambda c, vs: Wj_sb[:, c * V : (c + 1) * V][:, vs]
                acp_ps = {}
                for h in range(2):
                    hs = slice(64 * h, 64 * h + 64)
                    for vh in range(2):
                        vs = slice(vh * 512, (vh + 1) * 512)
                        ps = pp.tile([128, 512], FP32, tag="pps")
                        acp_ps[(h, vh)] = ps
                        for mc in range(4):
                            nc.tensor.matmul(
                                ps[64:128, :],
                                tfT[mc][:, hs],
                                Wjc(mc, vs),
                                start=(mc == 0),
                                stop=(mc == 3),
                                tile_position=(0, 64),
                            )
                nmv = 0
                for h in range(2):
                    for vh in range(2):
                        vs = slice(vh * 512, (vh + 1) * 512)
                        ps = acp_ps[(h, vh)]
                        for mc in range(4):
                            nc.tensor.matmul(
                                ps[0:64, :],
                                tgT[mc][:],
                                Wjc(4 + mc, vs),
                                start=(mc == 0),
                                stop=False,
                            )
                        nc.tensor.matmul(
                            ps[0:64, :], ones1, bj_sb[:, vs], start=False, stop=True
                        )
                        if nmv % 2 == 0:
                            nc.vector.tensor_copy(ACP[h][:, vs], ps[:])
                        else:
                            nc.scalar.copy(ACP[h][:, vs], ps[:])
                        nmv += 1

            # ---- main loop: 64 output tiles of [128, 1024].
            # Output DMA groups: small first/last (latency), 2 MB mid
            # (throughput), alternating between the two HWDGE rings.
            GSIZES = [4] * 16
            with (
                tc.tile_pool(name="po", bufs=4, space="PSUM") as po,
                tc.tile_pool(name="ob", bufs=6) as ob,
            ):
                k = 0
                for gi, gs in enumerate(GSIZES):
                    grp = ob.tile([128, 4 * V], FP16, tag="grp")
                    for s_ in range(gs):
                        h, kk = k // 32, k % 32
                        psO = po.tile([128, V], FP32, tag="psO")
                        Lk = L_sb[:, kk * 128 : (kk + 1) * 128]
                        for vh in range(2):
                            vs = slice(vh * 512, (vh + 1) * 512)
                            nc.tensor.matmul(
                                psO[:, vs], Lk, ACP[h][:, vs], start=True, stop=True
                            )
                        dst = grp[:, s_ * V : (s_ + 1) * V]
                        if k % 2 == 0:
                            nc.vector.tensor_copy(dst, psO[:])
                        else:
                            nc.scalar.copy(dst, psO[:])
                        k += 1
                    g0 = (k - gs) * 128
                    deng = nc.scalar if gi % 3 == 2 else nc.sync
                    deng.dma_start(
                        out_d[g0 : g0 + gs * 128, :].rearrange(
                            "(s p) v -> p s v", p=128
                        ),
                        grp[:, 0 : gs * V],
                    )

    nc.compile()
    return nc


def _host_consts():
    """32 L_k selectors [128, 32*128] and ident+ones [128, 2*128].
    L_k: (L_k.T @ ACP)[j,:] = C[j%64,:] + A[64h + 2k + j//64, :]."""
    Lx = np.zeros((128, 32, 128), np.float16)
    j = np.arange(128)
    u = j % 64
    hi = j // 64
    Lx[u, :, j] = 1.0
    for kk in range(32):
        Lx[64 + 2 * kk + hi, kk, j] = 1.0
    cx = np.zeros((128, 2, 128), np.float16)
    cx[j, 0, j] = 1.0  # identity
    cx[0, 1, 0:U] = 1.0  # ones row
    return (
        np.ascontiguousarray(Lx.reshape(128, 32 * 128)),
        np.ascontiguousarray(cx.reshape(128, 2 * 128)),
    )


def kernel(f, g, We, be, Wd, bd, Wj, bj):
    global _L_CONSTS
    if _L_CONSTS is None:
        _L_CONSTS = _host_consts()
    if "nc" not in _cache:
        _cache["nc"] = _build_nc()
    nc = _cache["nc"]

    c16 = lambda x: np.ascontiguousarray(np.asarray(x), dtype=np.float16)
    f16, g16 = c16(f), c16(g)
    be32 = np.asarray(be, np.float32).reshape(4, 128).T
    bd32 = np.asarray(bd, np.float32).reshape(4, 128).T
    bias8 = np.ascontiguousarray(
        np.concatenate([be32, bd32], axis=1), dtype=np.float32
    )
    shared = {
        "We": c16(We),
        "Wd": c16(Wd),
        "Wj": c16(Wj),
        "bj": c16(bj).reshape(1, V),
        "bias8": bias8,
        "Lsel": _L_CONSTS[0],
    }
    in_maps = []
    for c in range(NCORES):
        b, th = c // 2, c % 2
        in_maps.append(
            {
                "f_c": np.ascontiguousarray(
                    np.concatenate(
                        [f16[b, th * TC : (th + 1) * TC, :], _L_CONSTS[1]], axis=1
                    )
                ),
                "g_c": np.ascontiguousarray(g16[b]),
                **shared,
            }
        )
    res = run_bass_kernel_spmd(nc, in_maps, list(range(NCORES)))
    kernel._last_results = res

    out = np.empty((B, T, U, V), np.float32)
    for c in range(NCORES):
        b, th = c // 2, c % 2
        out[b, th * TC : (th + 1) * TC] = res.results[c]["out"].reshape(TC, U, V)
    return out


# revision 14
# speedup vs baseline: 1.0173x; 1.0173x over previous
"""RNN-T JointNetwork kernel for 8 Trainium2 NeuronCores.

Math: out[b,t,u,:] = tanh(concat(fe[b,t], gd[b,u])) @ Wj + bj
with fe = f@We+be, gd = g@Wd+bd.

tanh is elementwise and the concat feeds one GEMM, so the joint GEMM
factorizes exactly:
    out[b,t,u,:] = A[b,t,:] + C[b,u,:]
    A = tanh(f@We+be) @ Wj[:Dm]
    C = tanh(g@Wd+bd) @ Wj[Dm:] + bj
leaving the kernel bound by the output write (fp16: 16 MB/core).

Sharding: core c owns (b = c//2, t-half = c%2) -> [128,64,V] output chunk.

On-core plan (fp16 data, fp32 PSUM):
  - f/g straight-DMA'd, transposed on PE (fp16 psum passthrough);
    feT/gdT on PE; tanh+bias on ACT -> tfT/tgT
  - ACP[h] (h = t-half of the core's 128 rows): [128,V] fp16 tile with
    partitions 0:64 = C rows (all u) and 64:128 = A rows (t in half h),
    computed into one PSUM tile with C (cols 0:64) and A (cols 64:128)
    matmuls interleaved so the two PE column-groups run concurrently.
  - main loop, per output tile k (= t-pair 2k,2k+1; 128 DRAM rows):
    psO = L_k.T @ ACP[h], where L_k is a host-built 0/1 stationary
    selecting (A row, C row) per output row -> A-broadcast + C-add in
    a single PE pass.  Movers alternate per tile: DVE / ACT copy
    PSUM->SBUF fp16.  Output DMA: 1 MB per 4 tiles.
Host: casts inputs to fp16, builds L/identity, upcasts output on gather.
"""

import sys

sys.path.insert(0, "/opt/trn_rl_repo")

import numpy as np

import concourse.bacc as bacc
import concourse.mybir as mybir
import concourse.tile as tile
from concourse.bass_utils import run_bass_kernel_spmd
B, T, U = 4, 256, 64
D = 512  # DE = DD = DM
V = 1024
TC = 128  # t rows per core
NCORES = 8
FP32 = mybir.dt.float32
FP16 = mybir.dt.float16
TANH = mybir.ActivationFunctionType.Tanh
WIDE = False  # N=1024 matmul per tile fails the walrus ISA check (1 bank/MM)

_cache = {}
_L_CONSTS = None


def _build_nc():
    nc = bacc.Bacc("TRN2", target_bir_lowering=False)

    f_d = nc.dram_tensor("f_c", [TC, D + 256], FP16, kind="ExternalInput")
    g_d = nc.dram_tensor("g_c", [U, D], FP16, kind="ExternalInput")
    We_d = nc.dram_tensor("We", [D, D], FP16, kind="ExternalInput")
    Wd_d = nc.dram_tensor("Wd", [D, D], FP16, kind="ExternalInput")
    Wj_d = nc.dram_tensor("Wj", [2 * D, V], FP16, kind="ExternalInput")
    L_d = nc.dram_tensor("Lsel", [128, 32 * 128], FP16, kind="ExternalInput")
    bias_d = nc.dram_tensor("bias8", [128, 8], FP32, kind="ExternalInput")
    bj_d = nc.dram_tensor("bj", [1, V], FP16, kind="ExternalInput")
    out_d = nc.dram_tensor("out", [TC * U, V], FP16, kind="ExternalOutput")

    with tile.TileContext(nc) as tc:
        with tc.tile_pool(name="wts", bufs=1) as wp:
            # ---- persistent SBUF ----
            fc_sb = wp.tile([TC, D + 256], FP16, tag="f")
            f_sb = fc_sb[:, 0:D]
            c_sb = fc_sb[:, D : D + 256]
            g_sb = wp.tile([U, D], FP16, tag="g")
            We_sb = wp.tile([128, 4 * D], FP16, tag="We")
            Wd_sb = wp.tile([128, 4 * D], FP16, tag="Wd")
            Wj_sb = wp.tile([128, 8 * V], FP16, tag="Wj")
            L_sb = wp.tile([128, 32 * 128], FP16, tag="Lsel")
            bias_sb = wp.tile([128, 8], FP32, tag="bias8")
            bj_sb = wp.tile([1, V], FP16, tag="bj")
            fT = [wp.tile([128, TC], FP16, tag=f"fT{c}", name=f"fT{c}") for c in range(4)]
            gT = [wp.tile([128, U], FP16, tag=f"gT{c}", name=f"gT{c}") for c in range(4)]
            tfT = [wp.tile([128, TC], FP16, tag=f"tfT{c}", name=f"tfT{c}") for c in range(4)]
            tgT = [wp.tile([128, U], FP16, tag=f"tgT{c}", name=f"tgT{c}") for c in range(4)]
            ACP = [wp.tile([128, V], FP16, tag=f"ACP{h}", name=f"ACP{h}") for h in range(2)]

            ident = c_sb[:, 0:128]
            ones1 = c_sb[0:1, 128 : 128 + U]
            fc_sb  # noqa: B018

            # ---- input DMAs: few big transfers, split across HWDGE rings
            # sync ring: f+consts, We, Wj v-half 0, L (outputs follow in FIFO)
            # scalar ring: g, Wd, bias, bj, Wj v-half 1
            nc.sync.dma_start(fc_sb[:], f_d[:])
            nc.sync.dma_start(
                We_sb[:], We_d.rearrange("(c p) m -> p c m", p=128)
            )
            nc.scalar.dma_start(g_sb[:], g_d[:])
            nc.scalar.dma_start(bias_sb[:], bias_d[:])
            nc.scalar.dma_start(bj_sb[:], bj_d[:])
            nc.scalar.dma_start(
                Wd_sb[:], Wd_d.rearrange("(c p) m -> p c m", p=128)
            )
            # Wj_sb layout: [128, c*V + v] (chunk-major); DMA by row-half:
            # A rows (chunks 0-3) on sync, C rows (4-7) on scalar, so the
            # A matmuls can start as soon as the first MB lands.
            nc.sync.dma_start(
                Wj_sb[:, 0 : 4 * V].rearrange("p (c v) -> p c v", c=4),
                Wj_d[0 : 4 * 128, :].rearrange("(c p) v -> p c v", p=128),
            )
            nc.scalar.dma_start(
                Wj_sb[:, 4 * V : 8 * V].rearrange("p (c v) -> p c v", c=4),
                Wj_d[4 * 128 : 8 * 128, :].rearrange("(c p) v -> p c v", p=128),
            )
            # L selectors: first 8 early (scalar), rest late (sync)
            nc.scalar.dma_start(L_sb[:, 0 : 8 * 128], L_d[:, 0 : 8 * 128])
            nc.sync.dma_start(L_sb[:, 8 * 128 :], L_d[:, 8 * 128 :])

            # ---- prologue ----
            with tc.tile_pool(name="pp", bufs=4, space="PSUM") as pp:
                # transposes on PE (fp16 passthrough), copies split DVE/ACT
                for c in range(4):
                    pt = pp.tile([128, U], FP16, tag="ppt")
                    nc.tensor.transpose(
                        pt[:], g_sb[:, c * 128 : (c + 1) * 128], ident[0:64, 0:64]
                    )
                    if c % 2 == 0:
                        nc.vector.tensor_copy(gT[c][:], pt[:])
                    else:
                        nc.scalar.copy(gT[c][:], pt[:])
                for c in range(4):
                    pt = pp.tile([128, TC], FP16, tag="ppt")
                    nc.tensor.transpose(
                        pt[:], f_sb[:, c * 128 : (c + 1) * 128], ident
                    )
                    if c % 2 == 0:
                        nc.vector.tensor_copy(fT[c][:], pt[:])
                    else:
                        nc.scalar.copy(fT[c][:], pt[:])
                # HAM keepalive: junk matmuls (dep: fc only) bridge the idle
                # gap until We/Wd land, so the real chain runs at 2.4 GHz
                for w in range(8):
                    pw = pp.tile([128, 128], FP32, tag="ppt")
                    nc.tensor.matmul(
                        pw[:], ident, c_sb[:, 0:128], start=True, stop=True
                    )

                for mc in range(4):
                    ms = slice(mc * 128, (mc + 1) * 128)
                    ps = pp.tile([128, TC], FP32, tag="pps")
                    for dc in range(4):
                        nc.tensor.matmul(
                            ps[:],
                            We_sb[:, dc * D : (dc + 1) * D][:, ms],
                            fT[dc][:],
                            start=(dc == 0),
                            stop=(dc == 3),
                        )
                    nc.scalar.activation(
                        tfT[mc][:], ps[:], TANH, bias=bias_sb[:, mc : mc + 1]
                    )
                for mc in range(4):
                    ms = slice(mc * 128, (mc + 1) * 128)
                    ps = pp.tile([128, U], FP32, tag="pps")
                    for dc in range(4):
                        nc.tensor.matmul(
                            ps[:],
                            Wd_sb[:, dc * D : (dc + 1) * D][:, ms],
                            gT[dc][:],
                            start=(dc == 0),
                            stop=(dc == 3),
                        )
                    nc.scalar.activation(
                        tgT[mc][:], ps[:], TANH, bias=bias_sb[:, 4 + mc : 5 + mc]
                    )

                # ACP[h]: partitions 0:64 = C (all u), 64:128 = A rows of
                # half h.  A matmuls (gated on the sync Wj half) run first
                # across all four psum tiles; C matmuls follow when the
                # scalar Wj half lands; copies fire per tile as C completes.
                Wjc = lambda c, vs: Wj_sb[:, c * V : (c + 1) * V][:, vs]
                acp_ps = {}
                for h in range(2):
                    hs = slice(64 * h, 64 * h + 64)
                    for vh in range(2):
                        vs = slice(vh * 512, (vh + 1) * 512)
                        ps = pp.tile([128, 512], FP32, tag="pps")
                        acp_ps[(h, vh)] = ps
                        for mc in range(4):
                            nc.tensor.matmul(
                                ps[64:128, :],
                                tfT[mc][:, hs],
                                Wjc(mc, vs),
                                start=(mc == 0),
                                stop=(mc == 3),
                                tile_position=(0, 64),
                            )
                nmv = 0
                for h in range(2):
                    for vh in range(2):
                        vs = slice(vh * 512, (vh + 1) * 512)
                        ps = acp_ps[(h, vh)]
                        for mc in range(4):
                            nc.tensor.matmul(
                                ps[0:64, :],
                                tgT[mc][:],
                                Wjc(4 + mc, vs),
                                start=(mc == 0),
                                stop=False,
                            )
                        nc.tensor.matmul(
                            ps[0:64, :], ones1, bj_sb[:, vs], start=False, stop=True
                        )
                        if nmv % 2 == 0:
                            nc.vector.tensor_copy(ACP[h][:, vs], ps[:])
                        else:
                            nc.scalar.copy(ACP[h][:, vs], ps[:])
                        nmv += 1

            # ---- main loop: 64 output tiles of [128, 1024].
            # Output DMA groups: small first/last (latency), 2 MB mid
            # (throughput), alternating between the two HWDGE rings.
            GSIZES = [4] * 16
            with (
                tc.tile_pool(name="po", bufs=4, space="PSUM") as po,
                tc.tile_pool(name="ob", bufs=6) as ob,
            ):
                k = 0
                for gi, gs in enumerate(GSIZES):
                    grp = ob.tile([128, 4 * V], FP16, tag="grp")
                    for s_ in range(gs):
                        h, kk = k // 32, k % 32
                        psO = po.tile([128, V], FP32, tag="psO")
                        Lk = L_sb[:, kk * 128 : (kk + 1) * 128]
                        for vh in range(2):
                            vs = slice(vh * 512, (vh + 1) * 512)
                            nc.tensor.matmul(
                                psO[:, vs], Lk, ACP[h][:, vs], start=True, stop=True
                            )
                        dst = grp[:, s_ * V : (s_ + 1) * V]
                        if k % 2 == 0:
                            nc.vector.tensor_copy(dst, psO[:])
                        else:
                            nc.scalar.copy(dst, psO[:])
                        k += 1
                    g0 = (k - gs) * 128
                    deng = nc.scalar if gi % 3 == 2 else nc.sync
                    deng.dma_start(
                        out_d[g0 : g0 + gs * 128, :].rearrange(
                            "(s p) v -> p s v", p=128
                        ),
                        grp[:, 0 : gs * V],
                    )

    nc.compile()
    return nc


def _host_consts():
    """32 L_k selectors [128, 32*128] and ident+ones [128, 2*128].
    L_k: (L_k.T @ ACP)[j,:] = C[j%64,:] + A[64h + 2k + j//64, :]."""
    Lx = np.zeros((128, 32, 128), np.float16)
    j = np.arange(128)
    u = j % 64
    hi = j // 64
    Lx[u, :, j] = 1.0
    for kk in range(32):
        Lx[64 + 2 * kk + hi, kk, j] = 1.0
    cx = np.zeros((128, 2, 128), np.float16)
    cx[j, 0, j] = 1.0  # identity
    cx[0, 1, 0:U] = 1.0  # ones row
    return (
        np.ascontiguousarray(Lx.reshape(128, 32 * 128)),
        np.ascontiguousarray(cx.reshape(128, 2 * 128)),
    )


def kernel(f, g, We, be, Wd, bd, Wj, bj):
    global _L_CONSTS
    if _L_CONSTS is None:
        _L_CONSTS = _host_consts()
    if "nc" not in _cache:
        _cache["nc"] = _build_nc()
    nc = _cache["nc"]

    c16 = lambda x: np.ascontiguousarray(np.asarray(x), dtype=np.float16)
    f16, g16 = c16(f), c16(g)
    be32 = np.asarray(be, np.float32).reshape(4, 128).T
    bd32 = np.asarray(bd, np.float32).reshape(4, 128).T
    bias8 = np.ascontiguousarray(
        np.concatenate([be32, bd32], axis=1), dtype=np.float32
    )
    shared = {
        "We": c16(We),
        "Wd": c16(Wd),
        "Wj": c16(Wj),
        "bj": c16(bj).reshape(1, V),
        "bias8": bias8,
        "Lsel": _L_CONSTS[0],
    }
    in_maps = []
    for c in range(NCORES):
        b, th = c // 2, c % 2
        in_maps.append(
            {
                "f_c": np.ascontiguousarray(
                    np.concatenate(
                        [f16[b, th * TC : (th + 1) * TC, :], _L_CONSTS[1]], axis=1
                    )
                ),
                "g_c": np.ascontiguousarray(g16[b]),
                **shared,
            }
        )
    res = run_bass_kernel_spmd(nc, in_maps, list(range(NCORES)))
    kernel._last_results = res

    out = np.empty((B, T, U, V), np.float32)
    for c in range(NCORES):
        b, th = c // 2, c % 2
        out[b, th * TC : (th + 1) * TC] = res.results[c]["out"].reshape(TC, U, V)
    return out
